# revision 14
# baseline (speedup 1.0000x reference)
"""v3: fp16 matmuls + DMA-transposed Q/K + exp split across ACT/DVE.

Per core (one kv head, G=4 q heads, B=2): S = K^T Q and O = P V run as fp16
matmuls on PE (the flops floor).  Q/K reach [d, seq] layout via gpsimd
cast-DMA (f32->f16) into staging + one dma_start_transpose per load -- no PE
transposes, no psum round-trip, no copy instructions.  The softmax exp is
split by group between ACT (true exp) and DVE (Schraudolph: one tensor_scalar
(mult,add) -> uint16 bits reinterpreted as fp16; RNE on hw, ~1.7% rms).
Normalization: DVE reciprocal [128,2] per Qi, ACT Copy-activation with
per-partition scale AP writes OS.  Stores ride SP/HWDGE.
"""
import numpy as np
import concourse.bass as bass
from concourse import mybir
from contextlib import ExitStack

F32 = mybir.dt.float32
F16 = mybir.dt.float16
U16 = mybir.dt.uint16
EXP = mybir.ActivationFunctionType.Exp
COPY = mybir.ActivationFunctionType.Copy
SCALE = float(1.0 / np.sqrt(128.0))

N_CORES = 8

# Schraudolph constants: bits16 = rne(S * A_SCH + B_SCH), value = 2^((b-15360)/1024)
C_TUNE = -59.0
A_SCH = float(1024.0 / np.log(2.0) * SCALE)
B_SCH = float(15 * 1024.0 + C_TUNE)
FD = 0.32  # fraction of exp groups on DVE (Schraudolph)


def build_attention_nc(SEQ=2048, B=2, G=4, fd=FD):
    D = 128
    T = SEQ // 128            # 16 seq tiles
    QC = 128                  # q per group (one tile; one psum-bank accum region)
    NQC = T                   # 16 q chunks
    KG = 8                    # kt chunks per group
    NKP = T // KG             # 2 k partitions
    W = KG * QC               # 1024 psum cols per group
    H = B * G                 # 8 heads
    NG = H * NQC * NKP        # 256 groups
    NQ = H * NQC              # 128 Qi (q tiles across heads)
    assert 3 * W + 2 * 512 <= 4096

    nc = bass.Bass()
    q_ext = nc.declare_dram_parameter("query", [SEQ, B, G, D], F32, isOutput=False)
    k_ext = nc.declare_dram_parameter("key", [SEQ, B, D], F32, isOutput=False)
    v_ext = nc.declare_dram_parameter("value", [SEQ, B, D], F32, isOutput=False)
    o_ext = nc.declare_dram_parameter("out", [SEQ, B, G, D], F32, isOutput=True)

    # loads in first-use order: K(b), Q(b,0..G-1) per batch
    loads = []
    for b in range(B):
        loads.append(("K", b, None))
        for g in range(G):
            loads.append(("Q", b, g))
    NL = len(loads)
    NSTAGE = 4

    def q_load_index(h):
        b, g = divmod(h, G)
        return b * (G + 1) + 1 + g

    def k_load_index(b):
        return b * (G + 1)

    # exp engine assignment: 'D' (DVE schraudolph) with weight fd else 'A'
    eng_of = []
    for e in range(NG):
        eng_of.append("D" if int((e + 1) * fd) > int(e * fd) else "A")

    def eidx(e):
        kp = e % NKP
        Qi = e // NKP
        qc = Qi % NQC
        h = Qi // NQC
        return h, qc, kp, Qi

    # ---------------- schedule pass: exact semaphore counters ----------------
    # PE positions
    pe = 0
    pe_after_S = {}
    pe_after_O = {}
    for e in range(NG):
        pe += KG                      # S instrs
        pe_after_S[e] = pe
        if e >= 2:
            pe += KG            # O instrs of e-2
            pe_after_O[e - 2] = pe
    pe += KG
    pe_after_O[NG - 2] = pe
    pe += KG
    pe_after_O[NG - 1] = pe

    # ACT / DVE streams: exp counts, recips, mults.
    # group loop emits: exp(e) on its engine; at e == 4*Qi+4: recip(Qi) on DVE;
    # at e == 4*Qi+5: mults(Qi) on ACT.  Tail flushed after loop.
    act_n = 0
    dve_n = 0
    expdone = {}
    recipdone = {}
    multdone = {}

    def emit_recip_count(Qi):
        nonlocal dve_n
        dve_n += 1
        recipdone[Qi] = dve_n

    def emit_mult_count(Qi):
        nonlocal act_n
        act_n += 1
        multdone[Qi] = act_n

    for e in range(NG):
        if eng_of[e] == "A":
            act_n += 1
            expdone[e] = act_n
        else:
            dve_n += 1
            expdone[e] = dve_n
        if e >= NKP + 1 and (e - NKP - 1) % NKP == 0:
            emit_recip_count((e - NKP - 1) // NKP)
        if e >= NKP + 3 and (e - NKP - 3) % NKP == 0:
            emit_mult_count((e - NKP - 3) // NKP)
    for Qi in range(NQ):
        if Qi not in recipdone:
            emit_recip_count(Qi)
        if Qi not in multdone:
            emit_mult_count(Qi)

    # ---------------- tensors ----------------
    qnat = [nc.alloc_sbuf_tensor(f"qnat{i}", [128, T * 128], F16)
            for i in range(NSTAGE)]
    KT = [nc.alloc_sbuf_tensor(f"KT{b}", [128, T * 128], F16) for b in range(B)]
    QT = [nc.alloc_sbuf_tensor(f"QT{h}", [128, T * 128], F16) for h in range(H)]
    VT = [nc.alloc_sbuf_tensor(f"VT{b}", [128, T * 132], F16) for b in range(B)]
    PT = [nc.alloc_sbuf_tensor(f"PT{s}", [128, W], F16) for s in range(3)]
    rsb = [nc.alloc_sbuf_tensor(f"rsb{s}", [128, 1], F32) for s in range(4)]
    OS = [nc.alloc_sbuf_tensor(f"OS{s}", [128, T * 128], F32) for s in range(2)]
    psum = nc.alloc_psum_tensor("psum", [128, 4096], F32)

    def spsum(s):
        return psum[:, s * W:(s + 1) * W]

    def opsum(buf):
        off = 3072 + buf * 512
        return psum[:, off:off + 129]

    with ExitStack() as ctx:
        sem_pe = ctx.enter_context(nc.semaphore("sem_pe"))
        sem_act = ctx.enter_context(nc.semaphore("sem_act"))
        sem_dve = ctx.enter_context(nc.semaphore("sem_dve"))
        sem_ld = [ctx.enter_context(nc.semaphore(f"sem_ld{i}"))
                  for i in range(NL)]
        sem_tr = [ctx.enter_context(nc.semaphore(f"sem_tr{i}"))
                  for i in range(NL)]
        sem_v = [ctx.enter_context(nc.semaphore(f"sem_v{b}")) for b in range(B)]
        sem_out = [ctx.enter_context(nc.semaphore(f"sem_out{h}"))
                   for h in range(H)]
        block = ctx.enter_context(nc.Block())

        # -------- gpsimd: cast loads (f32 -> f16) for Q/K/V, ones cols ------
        @block.gpsimd
        def _(gp):
            for b in range(B):
                vt3 = VT[b][:].rearrange("p (t c) -> p t c", c=132)
                nc.gpsimd.memset(vt3[:, :, 128:129], 1.0)
                nc.gpsimd.dma_start(
                    out=vt3[:, :, 0:128],
                    in_=v_ext[:, b, :].rearrange("(t p) d -> p t d", p=128),
                ).then_inc(sem_v[b], 16)
            for i, (kind, b, g) in enumerate(loads):
                if i >= NSTAGE:
                    nc.gpsimd.wait_ge(sem_tr[i - NSTAGE], 16)
                src = k_ext[:, b, :] if kind == "K" else q_ext[:, b, g, :]
                nc.gpsimd.dma_start(
                    out=qnat[i % NSTAGE][:].rearrange("p (t d) -> p t d", d=128),
                    in_=src.rearrange("(t p) d -> p t d", p=128),
                ).then_inc(sem_ld[i], 16)

        # -------- SP: dma transposes + output stores ------------------------
        @block.sync
        def _(sync):
            for i, (kind, b, g) in enumerate(loads):
                nc.sync.wait_ge(sem_ld[i], 16)
                dst = KT[b] if kind == "K" else QT[b * G + g]
                nc.sync.dma_start_transpose(
                    out=dst[:].rearrange("p (t s) -> p t s", s=128),
                    in_=qnat[i % NSTAGE][:],
                ).then_inc(sem_tr[i], 16)
            for h in range(H):
                b, g = divmod(h, G)
                half = NQC // 2
                oh = o_ext[:, b, g, :].rearrange("(t p) d -> p t d", p=128)
                osh = OS[h % 2][:].rearrange("p (t d) -> p t d", d=128)
                nc.sync.wait_ge(sem_act, multdone[h * NQC + half - 1])
                nc.sync.dma_start(
                    out=oh[:, 0:T // 2, :], in_=osh[:, 0:T // 2, :],
                ).then_inc(sem_out[h], 16)
                nc.sync.wait_ge(sem_act, multdone[h * NQC + NQC - 1])
                nc.sync.dma_start(
                    out=oh[:, T // 2:T, :], in_=osh[:, T // 2:T, :],
                ).then_inc(sem_out[h], 16)

        # -------- PE: S and O matmuls ---------------------------------------
        @block.tensor
        def _(te):
            seen_tr = set()

            def need_tr(i):
                if i not in seen_tr:
                    seen_tr.add(i)
                    nc.tensor.wait_ge(sem_tr[i], 16)

            def emit_S(e):
                h, qc, kp, Qi = eidx(e)
                b = h // G
                s = e % 3
                need_tr(k_load_index(b))
                need_tr(q_load_index(h))
                w_exp = expdone.get(e - 3)
                w_sem = (sem_act if eng_of[e - 3] == "A" else sem_dve) \
                    if e >= 3 else None
                for ki in range(KG):
                    kt = kp * KG + ki
                    inst = nc.tensor.matmul(
                        spsum(s)[:, ki * QC:(ki + 1) * QC],
                        KT[b][:, kt * 128:(kt + 1) * 128],
                        QT[h][:, qc * QC:(qc + 1) * QC],
                        start=True, stop=True, skip_group_check=True,
                    )
                    if ki == 0 and w_sem is not None:
                        inst._wait_ge(w_sem, w_exp)
                    inst.then_inc(sem_pe)

            def emit_O(e):
                h, qc, kp, Qi = eidx(e)
                b = h // G
                s = e % 3
                buf = Qi % 2
                if kp == 0:
                    if Qi >= 2:
                        nc.tensor.wait_ge(sem_act, multdone[Qi - 2])
                        nc.tensor.wait_ge(sem_dve, recipdone[Qi - 2])
                    if e == b * G * NQC * NKP:
                        nc.tensor.wait_ge(sem_v[b], 16)
                vt3 = VT[b][:].rearrange("p (t c) -> p t c", c=132)
                for ki in range(KG):
                    kt = kp * KG + ki
                    inst = nc.tensor.matmul(
                        opsum(buf),
                        PT[s][:, ki * QC:ki * QC + 128],
                        vt3[:, kt, 0:129],
                        start=(kt == 0), stop=(kt == T - 1),
                        skip_group_check=True,
                    )
                    if ki == 0:
                        inst._wait_ge(
                            sem_act if eng_of[e] == "A" else sem_dve,
                            expdone[e])
                    inst.then_inc(sem_pe)

            for e in range(NG):
                emit_S(e)
                if e >= 2:
                    emit_O(e - 2)
            emit_O(NG - 2)
            emit_O(NG - 1)

        # -------- ACT: exp (true) + normalization mults ---------------------
        @block.scalar
        def _(sc):
            def emit_exp(e):
                s = e % 3
                nc.scalar.activation(
                    out=PT[s][:, 0:W], in_=spsum(s),
                    func=EXP, bias=0.0, scale=SCALE,
                )._wait_ge(sem_pe, pe_after_S[e]).then_inc(sem_act)

            def emit_mults(Qi):
                h, qc = divmod(Qi, NQC)
                buf = Qi % 2
                if qc == 0 and h >= 2:
                    nc.scalar.wait_ge(sem_out[h - 2], 32)
                nc.scalar.activation(
                    out=OS[h % 2][:, qc * 128:(qc + 1) * 128],
                    in_=opsum(buf)[:, 0:128],
                    func=COPY, bias=0.0,
                    scale=rsb[Qi % 4][:, 0:1],
                )._wait_ge(sem_dve, recipdone[Qi]).then_inc(sem_act)

            for e in range(NG):
                if eng_of[e] == "A":
                    emit_exp(e)
                if e >= NKP + 3 and (e - NKP - 3) % NKP == 0:
                    emit_mults((e - NKP - 3) // NKP)
            for Qi in range(NQ):
                if Qi * NKP + NKP + 3 > NG - 1:
                    emit_mults(Qi)

        # -------- DVE: schraudolph exp + reciprocals ------------------------
        @block.vector
        def _(ve):
            def emit_exp(e):
                s = e % 3
                nc.vector.tensor_scalar(
                    PT[s][:].bitcast(U16)[:, 0:W], spsum(s),
                    A_SCH, B_SCH,
                    op0=mybir.AluOpType.mult, op1=mybir.AluOpType.add,
                )._wait_ge(sem_pe, pe_after_S[e]).then_inc(sem_dve)

            def emit_recip(Qi):
                buf = Qi % 2
                if Qi >= 4:
                    nc.vector.wait_ge(sem_act, multdone[Qi - 4])
                nc.vector.reciprocal(
                    rsb[Qi % 4][:, 0:1], opsum(buf)[:, 128:129],
                )._wait_ge(sem_pe, pe_after_O[Qi * NKP + NKP - 1]
                           ).then_inc(sem_dve)

            for e in range(NG):
                if eng_of[e] == "D":
                    emit_exp(e)
                if e >= NKP + 1 and (e - NKP - 1) % NKP == 0:
                    emit_recip((e - NKP - 1) // NKP)
            for Qi in range(NQ):
                if Qi * NKP + NKP + 1 > NG - 1:
                    emit_recip(Qi)

    return nc


_NC = None


def _get_nc():
    global _NC
    if _NC is None:
        _NC = build_attention_nc(2048, 2, 4)
    return _NC


def kernel(query, key, value):
    from concourse.bass_utils import run_bass_kernel_spmd

    query = np.ascontiguousarray(query, dtype=np.float32)
    key = np.ascontiguousarray(key, dtype=np.float32)
    value = np.ascontiguousarray(value, dtype=np.float32)
    G = query.shape[2] // key.shape[2]
    nc = _get_nc()
    in_maps = []
    for c in range(N_CORES):
        in_maps.append({
            "query": np.ascontiguousarray(query[:, :, c * G:(c + 1) * G, :]),
            "key": np.ascontiguousarray(key[:, :, c, :]),
            "value": np.ascontiguousarray(value[:, :, c, :]),
        })
    res = run_bass_kernel_spmd(nc, in_maps, list(range(N_CORES)))
    out = np.empty_like(query)
    for c in range(N_CORES):
        out[:, :, c * G:(c + 1) * G, :] = res.results[c]["out"]
    return out


# revision 21
# speedup vs baseline: 1.3652x; 1.3652x over previous
"""v3: fp16 matmuls + DMA-transposed Q/K + exp split across ACT/DVE.

Per core (one kv head, G=4 q heads, B=2): S = K^T Q and O = P V run as fp16
matmuls on PE (the flops floor).  Q/K reach [d, seq] layout via gpsimd
cast-DMA (f32->f16) into staging + one dma_start_transpose per load -- no PE
transposes, no psum round-trip, no copy instructions.  The softmax exp is
split by group between ACT (true exp) and DVE (Schraudolph: one tensor_scalar
(mult,add) -> uint16 bits reinterpreted as fp16; RNE on hw, ~1.7% rms).
Normalization: DVE reciprocal [128,2] per Qi, ACT Copy-activation with
per-partition scale AP writes OS.  Stores ride SP/HWDGE.
"""
import numpy as np
import concourse.bass as bass
from concourse import mybir
from contextlib import ExitStack

F32 = mybir.dt.float32
F16 = mybir.dt.float16
U16 = mybir.dt.uint16
EXP = mybir.ActivationFunctionType.Exp
COPY = mybir.ActivationFunctionType.Copy
SCALE = float(1.0 / np.sqrt(128.0))

N_CORES = 8

# Schraudolph constants: bits16 = rne(S * A_SCH + B_SCH), value = 2^((b-15360)/1024)
C_TUNE = -59.0
A_SCH = float(1024.0 / np.log(2.0) * SCALE)
B_SCH = float(15 * 1024.0 + C_TUNE)
FD = 0.35  # fraction of exp groups on DVE (Schraudolph)


def build_attention_nc(SEQ=2048, B=2, G=4, fd=FD):
    D = 128
    T = SEQ // 128            # 16 seq tiles
    QC = 128                  # q per group (one tile; one psum-bank accum region)
    NQC = T                   # 16 q chunks
    KG = 8                    # kt chunks per group
    NKP = T // KG             # 2 k partitions
    W = KG * QC               # 1024 psum cols per group
    H = B * G                 # 8 heads
    NG = H * NQC * NKP        # 256 groups
    NQ = H * NQC              # 128 Qi (q tiles across heads)
    assert 3 * W + 2 * 512 <= 4096

    nc = bass.Bass()
    q_ext = nc.declare_dram_parameter("query", [SEQ, B, G, D], F32, isOutput=False)
    k_ext = nc.declare_dram_parameter("key", [SEQ, B, D], F32, isOutput=False)
    v_ext = nc.declare_dram_parameter("value", [SEQ, B, D], F32, isOutput=False)
    o_ext = nc.declare_dram_parameter("out", [SEQ, B, G, D], F32, isOutput=True)

    # loads in first-use order: K(b), Q(b,0..G-1) per batch
    loads = []
    for b in range(B):
        loads.append(("K", b, None))
        for g in range(G):
            loads.append(("Q", b, g))
    NL = len(loads)
    NSTAGE = 4

    def q_load_index(h):
        b, g = divmod(h, G)
        return b * (G + 1) + 1 + g

    def k_load_index(b):
        return b * (G + 1)

    # exp engine assignment: 'D' (DVE schraudolph) with weight fd else 'A'
    eng_of = []
    for e in range(NG):
        eng_of.append("D" if int((e + 1) * fd) > int(e * fd) else "A")

    def eidx(e):
        kp = e % NKP
        Qi = e // NKP
        qc = Qi % NQC
        h = Qi // NQC
        return h, qc, kp, Qi

    # ---------------- schedule pass: exact semaphore counters ----------------
    # PE positions
    pe = 0
    pe_after_S = {}
    pe_after_O = {}
    for e in range(NG):
        pe += KG                      # S instrs
        pe_after_S[e] = pe
        if e >= 2:
            pe += KG            # O instrs of e-2
            pe_after_O[e - 2] = pe
    pe += KG
    pe_after_O[NG - 2] = pe
    pe += KG
    pe_after_O[NG - 1] = pe

    # ACT / DVE streams: exp counts, recips, mults.
    # group loop emits: exp(e) on its engine; at e == 4*Qi+4: recip(Qi) on DVE;
    # at e == 4*Qi+5: mults(Qi) on ACT.  Tail flushed after loop.
    act_n = 0
    dve_n = 0
    expdone = {}
    recipdone = {}
    multdone = {}

    def emit_norm_count(Qi):
        nonlocal dve_n
        dve_n += 1
        recipdone[Qi] = dve_n
        dve_n += 1
        multdone[Qi] = dve_n

    for e in range(NG):
        if eng_of[e] == "A":
            act_n += 1
            expdone[e] = act_n
        else:
            dve_n += 1
            expdone[e] = dve_n
        if e >= NKP + 1 and (e - NKP - 1) % NKP == 0:
            emit_norm_count((e - NKP - 1) // NKP)
    for Qi in range(NQ):
        if Qi not in recipdone:
            emit_norm_count(Qi)

    # ---------------- tensors ----------------
    qnat = [nc.alloc_sbuf_tensor(f"qnat{i}", [128, T * 128], F16)
            for i in range(NSTAGE)]
    KT = [nc.alloc_sbuf_tensor(f"KT{b}", [128, T * 128], F16) for b in range(B)]
    QT = [nc.alloc_sbuf_tensor(f"QT{h}", [128, T * 128], F16) for h in range(H)]
    VT = [nc.alloc_sbuf_tensor(f"VT{b}", [128, T * 132], F16) for b in range(B)]
    PT = [nc.alloc_sbuf_tensor(f"PT{s}", [128, W], F16) for s in range(3)]
    rsb = [nc.alloc_sbuf_tensor(f"rsb{s}", [128, 1], F32) for s in range(4)]
    OS = [nc.alloc_sbuf_tensor(f"OS{s}", [128, T * 128], F32) for s in range(2)]
    psum = nc.alloc_psum_tensor("psum", [128, 4096], F32)

    def spsum(s):
        return psum[:, s * W:(s + 1) * W]

    def opsum(buf):
        off = 3072 + buf * 512
        return psum[:, off:off + 129]

    with ExitStack() as ctx:
        sem_pe = ctx.enter_context(nc.semaphore("sem_pe"))
        sem_act = ctx.enter_context(nc.semaphore("sem_act"))
        sem_dve = ctx.enter_context(nc.semaphore("sem_dve"))
        sem_ld = [ctx.enter_context(nc.semaphore(f"sem_ld{i}"))
                  for i in range(NL)]
        sem_tr = [ctx.enter_context(nc.semaphore(f"sem_tr{i}"))
                  for i in range(NL)]
        sem_v = [ctx.enter_context(nc.semaphore(f"sem_v{b}")) for b in range(B)]
        sem_out = [ctx.enter_context(nc.semaphore(f"sem_out{h}"))
                   for h in range(H)]
        block = ctx.enter_context(nc.Block())

        # -------- gpsimd: cast loads (f32 -> f16) for Q/K/V, ones cols ------
        @block.gpsimd
        def _(gp):
            for b in range(B):
                vt3 = VT[b][:].rearrange("p (t c) -> p t c", c=132)
                nc.gpsimd.memset(vt3[:, :, 128:129], 1.0)
                nc.gpsimd.dma_start(
                    out=vt3[:, :, 0:128],
                    in_=v_ext[:, b, :].rearrange("(t p) d -> p t d", p=128),
                ).then_inc(sem_v[b], 16)
            for i, (kind, b, g) in enumerate(loads):
                if i >= NSTAGE:
                    nc.gpsimd.wait_ge(sem_tr[i - NSTAGE], 16)
                src = k_ext[:, b, :] if kind == "K" else q_ext[:, b, g, :]
                nc.gpsimd.dma_start(
                    out=qnat[i % NSTAGE][:].rearrange("p (t d) -> p t d", d=128),
                    in_=src.rearrange("(t p) d -> p t d", p=128),
                ).then_inc(sem_ld[i], 16)

        # -------- SP: dma transposes + output stores ------------------------
        @block.sync
        def _(sync):
            for i, (kind, b, g) in enumerate(loads):
                nc.sync.wait_ge(sem_ld[i], 16)
                dst = KT[b] if kind == "K" else QT[b * G + g]
                nc.sync.dma_start_transpose(
                    out=dst[:].rearrange("p (t s) -> p t s", s=128),
                    in_=qnat[i % NSTAGE][:],
                ).then_inc(sem_tr[i], 16)
            for h in range(H):
                b, g = divmod(h, G)
                half = NQC // 2
                oh = o_ext[:, b, g, :].rearrange("(t p) d -> p t d", p=128)
                osh = OS[h % 2][:].rearrange("p (t d) -> p t d", d=128)
                nc.sync.wait_ge(sem_dve, multdone[h * NQC + half - 1])
                nc.sync.dma_start(
                    out=oh[:, 0:T // 2, :], in_=osh[:, 0:T // 2, :],
                ).then_inc(sem_out[h], 16)
                nc.sync.wait_ge(sem_dve, multdone[h * NQC + NQC - 1])
                nc.sync.dma_start(
                    out=oh[:, T // 2:T, :], in_=osh[:, T // 2:T, :],
                ).then_inc(sem_out[h], 16)

        # -------- PE: S and O matmuls ---------------------------------------
        @block.tensor
        def _(te):
            seen_tr = set()

            def need_tr(i):
                if i not in seen_tr:
                    seen_tr.add(i)
                    nc.tensor.wait_ge(sem_tr[i], 16)

            def emit_S(e):
                h, qc, kp, Qi = eidx(e)
                b = h // G
                s = e % 3
                need_tr(k_load_index(b))
                need_tr(q_load_index(h))
                w_exp = expdone.get(e - 3)
                w_sem = (sem_act if eng_of[e - 3] == "A" else sem_dve) \
                    if e >= 3 else None
                for ki in range(KG):
                    kt = kp * KG + ki
                    inst = nc.tensor.matmul(
                        spsum(s)[:, ki * QC:(ki + 1) * QC],
                        KT[b][:, kt * 128:(kt + 1) * 128],
                        QT[h][:, qc * QC:(qc + 1) * QC],
                        start=True, stop=True, skip_group_check=True,
                    )
                    if ki == 0 and w_sem is not None:
                        inst._wait_ge(w_sem, w_exp)
                    inst.then_inc(sem_pe)

            def emit_O(e):
                h, qc, kp, Qi = eidx(e)
                b = h // G
                s = e % 3
                buf = Qi % 2
                if kp == 0:
                    if Qi >= 2:
                        nc.tensor.wait_ge(sem_dve, multdone[Qi - 2])
                    if e == b * G * NQC * NKP:
                        nc.tensor.wait_ge(sem_v[b], 16)
                vt3 = VT[b][:].rearrange("p (t c) -> p t c", c=132)
                for ki in range(KG):
                    kt = kp * KG + ki
                    inst = nc.tensor.matmul(
                        opsum(buf),
                        PT[s][:, ki * QC:ki * QC + 128],
                        vt3[:, kt, 0:129],
                        start=(kt == 0), stop=(kt == T - 1),
                        skip_group_check=True,
                    )
                    if ki == 0:
                        inst._wait_ge(
                            sem_act if eng_of[e] == "A" else sem_dve,
                            expdone[e])
                    inst.then_inc(sem_pe)

            for e in range(NG):
                emit_S(e)
                if e >= 2:
                    emit_O(e - 2)
            emit_O(NG - 2)
            emit_O(NG - 1)

        # -------- ACT: exp (true) + normalization mults ---------------------
        @block.scalar
        def _(sc):
            def emit_exp(e):
                s = e % 3
                nc.scalar.activation(
                    out=PT[s][:, 0:W], in_=spsum(s),
                    func=EXP, bias=0.0, scale=SCALE,
                )._wait_ge(sem_pe, pe_after_S[e]).then_inc(sem_act)

            for e in range(NG):
                if eng_of[e] == "A":
                    emit_exp(e)

        # -------- DVE: schraudolph exp + reciprocals ------------------------
        @block.vector
        def _(ve):
            def emit_exp(e):
                s = e % 3
                nc.vector.tensor_scalar(
                    PT[s][:].bitcast(U16)[:, 0:W], spsum(s),
                    A_SCH, B_SCH,
                    op0=mybir.AluOpType.mult, op1=mybir.AluOpType.add,
                )._wait_ge(sem_pe, pe_after_S[e]).then_inc(sem_dve)

            def emit_norm(Qi):
                h, qc = divmod(Qi, NQC)
                buf = Qi % 2
                nc.vector.reciprocal(
                    rsb[Qi % 4][:, 0:1], opsum(buf)[:, 128:129],
                )._wait_ge(sem_pe, pe_after_O[Qi * NKP + NKP - 1]
                           ).then_inc(sem_dve)
                if qc == 0 and h >= 2:
                    nc.vector.wait_ge(sem_out[h - 2], 32)
                # self-wait drains the recip's rsb write (RAW across the
                # engine pipeline) before the scalar operand is fetched
                nc.vector.tensor_scalar(
                    OS[h % 2][:, qc * 128:(qc + 1) * 128],
                    opsum(buf)[:, 0:128],
                    rsb[Qi % 4][:, 0:1],
                    None,
                    op0=mybir.AluOpType.mult,
                )._wait_ge(sem_dve, recipdone[Qi]).then_inc(sem_dve)

            for e in range(NG):
                if eng_of[e] == "D":
                    emit_exp(e)
                if e >= NKP + 1 and (e - NKP - 1) % NKP == 0:
                    emit_norm((e - NKP - 1) // NKP)
            for Qi in range(NQ):
                if Qi * NKP + NKP + 1 > NG - 1:
                    emit_norm(Qi)

    return nc


_NC = None


def _get_nc():
    global _NC
    if _NC is None:
        _NC = build_attention_nc(2048, 2, 4)
    return _NC


def kernel(query, key, value):
    from concourse.bass_utils import run_bass_kernel_spmd

    query = np.ascontiguousarray(query, dtype=np.float32)
    key = np.ascontiguousarray(key, dtype=np.float32)
    value = np.ascontiguousarray(value, dtype=np.float32)
    G = query.shape[2] // key.shape[2]
    nc = _get_nc()
    in_maps = []
    for c in range(N_CORES):
        in_maps.append({
            "query": np.ascontiguousarray(query[:, :, c * G:(c + 1) * G, :]),
            "key": np.ascontiguousarray(key[:, :, c, :]),
            "value": np.ascontiguousarray(value[:, :, c, :]),
        })
    res = run_bass_kernel_spmd(nc, in_maps, list(range(N_CORES)))
    out = np.empty_like(query)
    for c in range(N_CORES):
        out[:, :, c * G:(c + 1) * G, :] = res.results[c]["out"]
    return out


# revision 33
# speedup vs baseline: 1.4553x; 1.0660x over previous
"""v3: fp16 matmuls + DMA-transposed Q/K + exp split across ACT/DVE.

Per core (one kv head, G=4 q heads, B=2): S = K^T Q and O = P V run as fp16
matmuls on PE (the flops floor).  Q/K reach [d, seq] layout via gpsimd
cast-DMA (f32->f16) into staging + one dma_start_transpose per load -- no PE
transposes, no psum round-trip, no copy instructions.  The softmax exp is
split by group between ACT (true exp) and DVE (Schraudolph: one tensor_scalar
(mult,add) -> uint16 bits reinterpreted as fp16; RNE on hw, ~1.7% rms).
Normalization: DVE reciprocal [128,2] per Qi, ACT Copy-activation with
per-partition scale AP writes OS.  Stores ride SP/HWDGE.
"""
import numpy as np
import concourse.bass as bass
from concourse import mybir
from contextlib import ExitStack

F32 = mybir.dt.float32
F16 = mybir.dt.float16
U16 = mybir.dt.uint16
EXP = mybir.ActivationFunctionType.Exp
COPY = mybir.ActivationFunctionType.Copy
SCALE = float(1.0 / np.sqrt(128.0))

N_CORES = 8

# Schraudolph constants: bits16 = rne(S * A_SCH + B_SCH), value = 2^((b-15360)/1024)
C_TUNE = -59.0
A_SCH = float(1024.0 / np.log(2.0) * SCALE)
B_SCH = float(15 * 1024.0 + C_TUNE)
FD = 0.35  # fraction of exp groups on DVE (Schraudolph)


def build_attention_nc(SEQ=2048, B=2, G=4, fd=FD):
    D = 128
    T = SEQ // 128            # 16 seq tiles
    QC = 128                  # q per group (one tile; one psum-bank accum region)
    NQC = T                   # 16 q chunks
    KG = 8                    # kt chunks per group
    NKP = T // KG             # 2 k partitions
    W = KG * QC               # 1024 psum cols per group
    H = B * G                 # 8 heads
    NG = H * NQC * NKP        # 256 groups
    NQ = H * NQC              # 128 Qi (q tiles across heads)
    assert 3 * W + 2 * 512 <= 4096

    nc = bass.Bass()
    q_ext = nc.declare_dram_parameter("query", [SEQ, B, G, D], F32, isOutput=False)
    k_ext = nc.declare_dram_parameter("key", [SEQ, B, D], F32, isOutput=False)
    v_ext = nc.declare_dram_parameter("value", [SEQ, B, D], F32, isOutput=False)
    o_ext = nc.declare_dram_parameter("out", [SEQ, B, G, D], F32, isOutput=True)

    # loads in first-use order: K(b), Q(b,0..G-1) per batch
    loads = []
    for b in range(B):
        loads.append(("K", b, None))
        for g in range(G):
            loads.append(("Q", b, g))
    NL = len(loads)
    NSTAGE = 6

    def q_load_index(h):
        b, g = divmod(h, G)
        return b * (G + 1) + 1 + g

    def k_load_index(b):
        return b * (G + 1)

    # exp engine assignment: 'D' (DVE schraudolph) with weight fd else 'A'
    eng_of = []
    for e in range(NG):
        eng_of.append("D" if int((e + 1) * fd) > int(e * fd) else "A")

    def eidx(e):
        kp = e % NKP
        Qi = e // NKP
        qc = Qi % NQC
        h = Qi // NQC
        return h, qc, kp, Qi

    # ---------------- schedule pass: exact semaphore counters ----------------
    # PE positions
    pe = 0
    pe_after_S = {}
    pe_after_O = {}
    for e in range(NG):
        pe += KG                      # S instrs
        pe_after_S[e] = pe
        if e >= 2:
            pe += KG            # O instrs of e-2
            pe_after_O[e - 2] = pe
    pe += KG
    pe_after_O[NG - 2] = pe
    pe += KG
    pe_after_O[NG - 1] = pe

    # ACT / DVE streams: exp counts, recips, mults.
    # group loop emits: exp(e) on its engine; at e == 4*Qi+4: recip(Qi) on DVE;
    # at e == 4*Qi+5: mults(Qi) on ACT.  Tail flushed after loop.
    act_n = 0
    dve_n = 0
    expdone = {}
    recipdone = {}
    multdone = {}

    def emit_norm_count(Qi):
        nonlocal dve_n
        dve_n += 1
        recipdone[Qi] = dve_n
        dve_n += 1
        multdone[Qi] = dve_n

    for e in range(NG):
        if eng_of[e] == "A":
            act_n += 1
            expdone[e] = act_n
        else:
            dve_n += 1
            expdone[e] = dve_n
        if e >= NKP + 1 and (e - NKP - 1) % NKP == 0:
            emit_norm_count((e - NKP - 1) // NKP)
    for Qi in range(NQ):
        if Qi not in recipdone:
            emit_norm_count(Qi)

    # ---------------- tensors ----------------
    qnat = [nc.alloc_sbuf_tensor(f"qnat{i}", [128, T * 128], F16)
            for i in range(NSTAGE)]
    KT = [nc.alloc_sbuf_tensor(f"KT{b}", [128, T * 128], F16) for b in range(B)]
    QT = [nc.alloc_sbuf_tensor(f"QT{h}", [128, T * 128], F16) for h in range(H)]
    VT = [nc.alloc_sbuf_tensor(f"VT{b}", [128, T * 132], F16) for b in range(B)]
    PT = [nc.alloc_sbuf_tensor(f"PT{s}", [128, W], F16) for s in range(3)]
    rsb = [nc.alloc_sbuf_tensor(f"rsb{s}", [128, 1], F32) for s in range(4)]
    OS = [nc.alloc_sbuf_tensor(f"OS{s}", [128, T * 128], F32) for s in range(2)]
    psum = nc.alloc_psum_tensor("psum", [128, 4096], F32)

    def spsum(s):
        return psum[:, s * W:(s + 1) * W]

    def opsum(buf):
        off = 3072 + buf * 512
        return psum[:, off:off + 129]

    with ExitStack() as ctx:
        sem_pe = ctx.enter_context(nc.semaphore("sem_pe"))
        sem_act = ctx.enter_context(nc.semaphore("sem_act"))
        sem_dve = ctx.enter_context(nc.semaphore("sem_dve"))
        sem_ld = [[ctx.enter_context(nc.semaphore(f"sem_ld{i}_{h}"))
                   for h in range(2)] for i in range(NL)]
        sem_tr = [[ctx.enter_context(nc.semaphore(f"sem_tr{i}_{h}"))
                   for h in range(2)] for i in range(NL)]
        sem_v = [[ctx.enter_context(nc.semaphore(f"sem_v{b}_{h}"))
                  for h in range(2)] for b in range(B)]
        sem_out = [ctx.enter_context(nc.semaphore(f"sem_out{h}"))
                   for h in range(H)]
        block = ctx.enter_context(nc.Block())

        # -------- gpsimd: cast loads (f32 -> f16) for Q/K/V, ones cols ------
        # All loads split in seq-halves so the SP transposes can interleave
        # with them on the (serial) DMA device; triggers are sem-gated to
        # control the enqueue order during the prologue.
        TH = T // 2

        @block.gpsimd
        def _(gp):
            def load_qk(i, h, gate=None):
                kind, b, g = loads[i]
                if gate is not None:
                    nc.gpsimd.wait_ge(gate, 16)
                if i >= NSTAGE:
                    nc.gpsimd.wait_ge(sem_tr[i - NSTAGE][h], 16)
                src = k_ext[:, b, :] if kind == "K" else q_ext[:, b, g, :]
                nc.gpsimd.dma_start(
                    out=qnat[i % NSTAGE][:].rearrange(
                        "p (t d) -> p t d", d=128)[:, h * TH:(h + 1) * TH, :],
                    in_=src.rearrange(
                        "(t p) d -> p t d", p=128)[:, h * TH:(h + 1) * TH, :],
                ).then_inc(sem_ld[i][h], 16)

            def load_v(b, h, gate=None):
                if gate is not None:
                    nc.gpsimd.wait_ge(gate, 16)
                vt3 = VT[b][:].rearrange("p (t c) -> p t c", c=132)
                nc.gpsimd.dma_start(
                    out=vt3[:, h * TH:(h + 1) * TH, 0:128],
                    in_=v_ext[:, b, :].rearrange(
                        "(t p) d -> p t d", p=128)[:, h * TH:(h + 1) * TH, :],
                ).then_inc(sem_v[b][h], 16)

            # prologue: K0a, Q0a first; K0b enqueues after the trK0a/trQ0a
            # triggers; V0 and Q0b follow K0b.
            load_qk(0, 0)
            load_qk(1, 0)
            for b in range(B):
                vt3 = VT[b][:].rearrange("p (t c) -> p t c", c=132)
                nc.gpsimd.memset(vt3[:, :, 128:129], 1.0)
            load_qk(0, 1)
            load_v(0, 0, gate=sem_ld[0][1])
            load_v(0, 1, gate=sem_ld[0][1])
            load_qk(1, 1, gate=sem_ld[0][1])
            for i in range(2, NL):
                load_qk(i, 0, gate=sem_tr[i - 2][0])
                load_qk(i, 1, gate=sem_tr[i - 2][1])
                if i == 4:
                    load_v(1, 0, gate=sem_tr[2][0])
                    load_v(1, 1, gate=sem_tr[2][1])

        # -------- SP: dma transposes + output stores ------------------------
        @block.sync
        def _(sync):
            tr_order = [(0, 0), (1, 0), (0, 1), (1, 1)]
            for i in range(2, NL):
                tr_order += [(i, 0), (i, 1)]
            for i, h in tr_order:
                kind, b, g = loads[i]
                dst = KT[b] if kind == "K" else QT[b * G + g]
                nc.sync.wait_ge(sem_ld[i][h], 16)
                nc.sync.dma_start_transpose(
                    out=dst[:].rearrange(
                        "p (t s) -> p t s", s=128)[:, h * TH:(h + 1) * TH, :],
                    in_=qnat[i % NSTAGE][:, h * TH * 128:(h + 1) * TH * 128],
                ).then_inc(sem_tr[i][h], 16)
            for h in range(H):
                b, g = divmod(h, G)
                oh = o_ext[:, b, g, :].rearrange("(t p) d -> p t d", p=128)
                osh = OS[h % 2][:].rearrange("p (t d) -> p t d", d=128)
                # last head: 4 store pieces so the drain tail stays short
                npieces = 4 if h == H - 1 else 2
                step = NQC // npieces
                inc = 32 // npieces if npieces == 2 else 16
                for p in range(npieces):
                    nc.sync.wait_ge(sem_dve,
                                    multdone[h * NQC + (p + 1) * step - 1])
                    nc.sync.dma_start(
                        out=oh[:, p * step:(p + 1) * step, :],
                        in_=osh[:, p * step:(p + 1) * step, :],
                    ).then_inc(sem_out[h], 16)

        # -------- PE: S and O matmuls ---------------------------------------
        @block.tensor
        def _(te):
            seen_tr = set()
            seen_v = set()

            def need_tr(i, half):
                if (i, half) not in seen_tr:
                    seen_tr.add((i, half))
                    nc.tensor.wait_ge(sem_tr[i][half], 16)

            def emit_S(e):
                h, qc, kp, Qi = eidx(e)
                b = h // G
                s = e % 3
                need_tr(k_load_index(b), kp)
                need_tr(q_load_index(h), qc // TH)
                w_exp = expdone.get(e - 3)
                w_sem = (sem_act if eng_of[e - 3] == "A" else sem_dve) \
                    if e >= 3 else None
                for ki in range(KG):
                    kt = kp * KG + ki
                    inst = nc.tensor.matmul(
                        spsum(s)[:, ki * QC:(ki + 1) * QC],
                        KT[b][:, kt * 128:(kt + 1) * 128],
                        QT[h][:, qc * QC:(qc + 1) * QC],
                        start=True, stop=True, skip_group_check=True,
                    )
                    if ki == 0 and w_sem is not None:
                        inst._wait_ge(w_sem, w_exp)
                    inst.then_inc(sem_pe)

            def emit_O(e):
                h, qc, kp, Qi = eidx(e)
                b = h // G
                s = e % 3
                buf = Qi % 2
                if kp == 0:
                    if Qi >= 2:
                        nc.tensor.wait_ge(sem_dve, multdone[Qi - 2])
                if (b, kp) not in seen_v:
                    seen_v.add((b, kp))
                    nc.tensor.wait_ge(sem_v[b][kp], 16)
                vt3 = VT[b][:].rearrange("p (t c) -> p t c", c=132)
                for ki in range(KG):
                    kt = kp * KG + ki
                    inst = nc.tensor.matmul(
                        opsum(buf),
                        PT[s][:, ki * QC:ki * QC + 128],
                        vt3[:, kt, 0:129],
                        start=(kt == 0), stop=(kt == T - 1),
                        skip_group_check=True,
                    )
                    if ki == 0:
                        inst._wait_ge(
                            sem_act if eng_of[e] == "A" else sem_dve,
                            expdone[e])
                    inst.then_inc(sem_pe)

            for e in range(NG):
                emit_S(e)
                if e >= 2:
                    emit_O(e - 2)
            emit_O(NG - 2)
            emit_O(NG - 1)

        # -------- ACT: exp (true) + normalization mults ---------------------
        @block.scalar
        def _(sc):
            def emit_exp(e):
                s = e % 3
                nc.scalar.activation(
                    out=PT[s][:, 0:W], in_=spsum(s),
                    func=EXP, bias=0.0, scale=SCALE,
                )._wait_ge(sem_pe, pe_after_S[e]).then_inc(sem_act)

            for e in range(NG):
                if eng_of[e] == "A":
                    emit_exp(e)

        # -------- DVE: schraudolph exp + reciprocals ------------------------
        @block.vector
        def _(ve):
            def emit_exp(e):
                s = e % 3
                nc.vector.tensor_scalar(
                    PT[s][:].bitcast(U16)[:, 0:W], spsum(s),
                    A_SCH, B_SCH,
                    op0=mybir.AluOpType.mult, op1=mybir.AluOpType.add,
                )._wait_ge(sem_pe, pe_after_S[e]).then_inc(sem_dve)

            def emit_norm(Qi):
                h, qc = divmod(Qi, NQC)
                buf = Qi % 2
                nc.vector.reciprocal(
                    rsb[Qi % 4][:, 0:1], opsum(buf)[:, 128:129],
                )._wait_ge(sem_pe, pe_after_O[Qi * NKP + NKP - 1]
                           ).then_inc(sem_dve)
                if qc == 0 and h >= 2:
                    nc.vector.wait_ge(sem_out[h - 2], 32)
                # self-wait drains the recip's rsb write (RAW across the
                # engine pipeline) before the scalar operand is fetched
                nc.vector.tensor_scalar(
                    OS[h % 2][:, qc * 128:(qc + 1) * 128],
                    opsum(buf)[:, 0:128],
                    rsb[Qi % 4][:, 0:1],
                    None,
                    op0=mybir.AluOpType.mult,
                )._wait_ge(sem_dve, recipdone[Qi]).then_inc(sem_dve)

            for e in range(NG):
                if eng_of[e] == "D":
                    emit_exp(e)
                if e >= NKP + 1 and (e - NKP - 1) % NKP == 0:
                    emit_norm((e - NKP - 1) // NKP)
            for Qi in range(NQ):
                if Qi * NKP + NKP + 1 > NG - 1:
                    emit_norm(Qi)

    return nc


_NC = None


def _get_nc():
    global _NC
    if _NC is None:
        _NC = build_attention_nc(2048, 2, 4)
    return _NC


def kernel(query, key, value):
    from concourse.bass_utils import run_bass_kernel_spmd

    query = np.ascontiguousarray(query, dtype=np.float32)
    key = np.ascontiguousarray(key, dtype=np.float32)
    value = np.ascontiguousarray(value, dtype=np.float32)
    G = query.shape[2] // key.shape[2]
    nc = _get_nc()
    in_maps = []
    for c in range(N_CORES):
        in_maps.append({
            "query": np.ascontiguousarray(query[:, :, c * G:(c + 1) * G, :]),
            "key": np.ascontiguousarray(key[:, :, c, :]),
            "value": np.ascontiguousarray(value[:, :, c, :]),
        })
    res = run_bass_kernel_spmd(nc, in_maps, list(range(N_CORES)))
    out = np.empty_like(query)
    for c in range(N_CORES):
        out[:, :, c * G:(c + 1) * G, :] = res.results[c]["out"]
    return out
